# revision 14
# baseline (speedup 1.0000x reference)
"""Distributed Trainium2 kernel for causal multi-head attention with LoRA
(c_attn + c_proj both LoRA'd), B=2 T=2048 C=1024 H=16 hd=64 r=8.

Sharding: data-parallel over batch (2 groups of 4 cores) x tensor-parallel
over heads (4 heads / core).  Each core computes qkv for its heads, causal
attention, and a partial c_proj over its 256 input dims; the host sums the
4 partial outputs per batch group.

Host-side simplifications (all exact linear algebra, no approximation):
 - LoRA folds into the base weights: W_eff = W + LORA_SCALE * B @ A.
 - Everything is passed feature-major ("pre-transposed") so no on-device
   transposes are needed; the device output is y^T, transposed back on host.
 - b_attn / b_proj are zeros by the problem spec and are not applied.

Device compute is bf16 (fp32 PSUM accumulation; rel-err budget 2e-2).

Schedule: attention chunks ascend (smallest first) so the ScalarE-heavy
softmax work of early chunks overlaps the PE-heavy qkv matmuls of later
k-tiles.  A queue of PE "filler" closures (qkv groups, v groups, c_proj
m-tiles of finished chunks) is drained between attention windows so the PE
never idles long enough for the HAM clock gate to re-throttle.

Engine placement: ScalarE does ONLY exp.  All PSUM->SBUF drains (qkv/v/o/
sums/y copies) are on VectorE.  Causal mask multiplies (SBUF-only, band-
trimmed to the 128-col partial-diagonal band) are on GpSimd.  Softmax
denominators use the augmented-V ones-column trick (no extra PE streams);
their reciprocal is DVE reciprocal_approx_fast.
"""

import numpy as np
import ml_dtypes

import concourse.bass as bass
import concourse.mybir as mybir
import concourse.tile as tile
from concourse import bacc

BF16 = mybir.dt.bfloat16
F32 = mybir.dt.float32
NPBF = ml_dtypes.bfloat16

B, T, C = 2, 2048, 1024
H, HD, R = 16, 64, 8
LORA_SCALE = 2.0

TP = 4                 # tensor-parallel ranks per batch group
HL = H // TP           # heads per core = 4
OQ = HL * HD           # local q rows = 256
OL = 3 * OQ            # local qkv rows = 768
CP = C // TP           # local c_proj contraction dims = 256
TC = 512               # t-chunk (matmul free dim)
NTC = T // TC          # 4 chunks
KT = 128               # k tile (partition dim of S^T)
NCT = C // 128         # 8 contraction tiles for c_attn

# attention chunks, ascending so softmax overlaps remaining qkv work
CHUNKS = [(0, 256), (256, 256), (512, 512), (1024, 512), (1536, 512)]


def build_nc():
    nc = bacc.Bacc(None, target_bir_lowering=False)

    xt_d = nc.declare_dram_parameter("xt", [C, T], BF16, isOutput=False)
    wqkvt_d = nc.declare_dram_parameter("wqkvt", [C, OL], BF16, isOutput=False)
    wpt_d = nc.declare_dram_parameter("wpt", [CP, C], BF16, isOutput=False)
    masks_d = nc.declare_dram_parameter("masks", [4, KT, TC], BF16, isOutput=False)
    out_d = nc.declare_dram_parameter("out", [C, T], BF16, isOutput=True)

    with tile.TileContext(nc) as tc:
        with (
            tc.tile_pool(name="const", bufs=1) as const,
            tc.tile_pool(name="work", bufs=3) as work,
            tc.tile_pool(name="ps_lin", bufs=2, space="PSUM") as ps_lin,
            tc.tile_pool(name="ps_s", bufs=1, space="PSUM") as ps_s,
            tc.tile_pool(name="ps_o", bufs=1, space="PSUM") as ps_o,
        ):
            # ---------------- persistent SBUF tensors ----------------
            wq_s = const.tile([128, NCT, OL], BF16, tag="wq")
            wq_r = wqkvt_d.rearrange("(n p) o -> p n o", p=128)
            for n in range(NCT):
                nc.sync.dma_start(out=wq_s[:, n, :], in_=wq_r[:, n, :])

            xt_s = const.tile([128, NCT, T], BF16, tag="xt")
            xt_r = xt_d.rearrange("(n p) t -> p n t", p=128)
            for n in range(NCT):
                nc.sync.dma_start(
                    out=xt_s[:, n, bass.ts(0, TC)],
                    in_=xt_r[:, n, bass.ts(0, TC)],
                )

            mask_s = const.tile([128, 4, TC], BF16, tag="mask")
            nc.sync.dma_start(out=mask_s, in_=masks_d.rearrange("j p q -> p j q"))

            for ci in range(1, NTC):
                for n in range(NCT):
                    nc.sync.dma_start(
                        out=xt_s[:, n, bass.ts(ci, TC)],
                        in_=xt_r[:, n, bass.ts(ci, TC)],
                    )

            wpt_s = const.tile([128, CP // 128, C], BF16, tag="wpt")
            nc.sync.dma_start(out=wpt_s, in_=wpt_d.rearrange("(n p) o -> p n o", p=128))

            # q,k feature-major: tiles 0,1 = q (256 rows), 2,3 = k
            qkvt_s = const.tile([128, 4, T], BF16, tag="qkvt")
            # v token-major, augmented: per t-tile, 4 heads x (64 dims + ones)
            v_s = const.tile([128, T // 128, HL * (HD + 1)], BF16, tag="v")
            nc.vector.memset(v_s, 1.0)  # ones columns survive the V copies
            ot_s = const.tile([128, CP // 128, T], BF16, tag="ot")
            ones_s = const.tile([128, 64], BF16, tag="ones")
            nc.vector.memset(ones_s, 1.0)

            # PE warmup: dummy matmuls during the input-DMA window so the
            # HAM clock gate reaches 8/8 before real work starts.  The memset
            # runs on GpSimd (whose queue is otherwise empty at t0) so the
            # warmup isn't blocked behind the DVE memset/TENSOR_LOAD chain.
            warm_s = const.tile([128, TC], BF16, tag="warm")
            nc.gpsimd.memset(warm_s, 0.0)
            warm_ps = ps_lin.tile([128, TC], F32, tag="lin", name="warm_ps")
            for _ in range(10):
                nc.tensor.matmul(
                    warm_ps, lhsT=warm_s[:, :128], rhs=warm_s,
                    start=True, stop=True,
                )

            # ---------------- PE filler emitters ----------------
            def qk_group(j, ci):
                # feature-major q/k: o-tile j (0,1=q pairs; 2,3=k pairs)
                osl = bass.ts(j, 128)
                tsl = bass.ts(ci, TC)
                qk_ps = ps_lin.tile([128, TC], F32, tag="lin", name="qk_ps")
                for n in range(NCT):
                    nc.tensor.matmul(
                        qk_ps, lhsT=wq_s[:, n, osl], rhs=xt_s[:, n, tsl],
                        start=(n == 0), stop=(n == NCT - 1),
                    )
                nc.vector.tensor_copy(qkvt_s[:, j, tsl], qk_ps)

            def v_group(tt):
                # v token-major (+ ones column per head)
                v_ps = ps_lin.tile([128, TC], F32, tag="lin", name="v_ps")
                ttsl = bass.ts(tt, 128)
                for n in range(NCT):
                    nc.tensor.matmul(
                        v_ps[:, :OQ], lhsT=xt_s[:, n, ttsl], rhs=wq_s[:, n, 2 * OQ:OL],
                        start=(n == 0), stop=(n == NCT - 1),
                    )
                dst = v_s[:, tt, :].rearrange("p (h e) -> p h e", e=HD + 1)[:, :, 0:HD]
                nc.vector.tensor_copy(dst, v_ps[:, :OQ].rearrange("p (h e) -> p h e", e=HD))

            # np (normalize + c_proj) sub-steps for a finished chunk
            def np_head(st):
                q0, qw, sums = st["q0"], st["qw"], st["sums"]
                recf = work.tile([128, TC], F32, tag="recf", name="recf")
                recip = work.tile([128, TC], BF16, tag="recip", name="recip")
                nc.vector.reciprocal_approx_fast(recf[:, :qw], sums[:, :qw])
                with nc.allow_low_precision(reason="softmax denom, 2e-2 budget"):
                    nc.vector.tensor_copy(recip[:, :qw], recf[:, :qw])
                tsl = slice(q0, q0 + qw)
                for p in range(2):
                    rb_ps = ps_lin.tile([128, TC], F32, tag="lin", name="rb_ps")
                    for h01 in range(2):
                        h = 2 * p + h01
                        nc.tensor.matmul(
                            rb_ps[64 * h01:64 * h01 + 64, :qw],
                            lhsT=ones_s[32 * h:32 * h + 1, :],
                            rhs=recip[32 * h:32 * h + 1, :qw],
                            start=True, stop=True,
                            tile_position=(32 * h, 64 * h01),
                        )
                    dst = ot_s[:, p, tsl]
                    nc.vector.tensor_mul(dst, dst, rb_ps[:, :qw])
                st["yt"] = work.tile(
                    [128, C // 128, TC], BF16, tag="yt", bufs=2, name="yt"
                )

            def np_y(st, m):
                q0, qw = st["q0"], st["qw"]
                tsl = slice(q0, q0 + qw)
                msl = bass.ts(m, 128)
                y_ps = ps_lin.tile([128, TC], F32, tag="lin", name="y_ps")
                for n in range(CP // 128):
                    nc.tensor.matmul(
                        y_ps[:, :qw], lhsT=wpt_s[:, n, msl], rhs=ot_s[:, n, tsl],
                        start=(n == 0), stop=(n == CP // 128 - 1),
                    )
                nc.vector.tensor_copy(st["yt"][:, m, :qw], y_ps[:, :qw])

            def np_dma(st, half):
                q0, qw = st["q0"], st["qw"]
                tsl = slice(q0, q0 + qw)
                out_r = out_d.rearrange("(m p) t -> p m t", p=128)
                nc.sync.dma_start(
                    out=out_r[:, 4 * half:4 * half + 4, tsl],
                    in_=st["yt"][:, 4 * half:4 * half + 4, :qw],
                )

            # Two filler queues: prereqs of upcoming attention chunks (gate
            # progress, drain first) and np work of finished chunks (no
            # downstream consumer; held until the late chunks where the PE
            # has slack inside exp-paced windows).
            filler_q = []
            np_q = []
            allow_np = [False]

            def push_np(st):
                np_q.append((900, lambda: np_head(st)))
                for m in range(C // 128):
                    np_q.append((500, lambda m=m: np_y(st, m)))
                np_q.append((200, lambda: np_dma(st, 0)))
                np_q.append((200, lambda: np_dma(st, 1)))

            def drain(ns):
                # emit fillers worth ~ns of PE time
                while filler_q and ns > 0:
                    cost, fn = filler_q.pop(0)
                    fn()
                    ns -= cost
                while allow_np[0] and np_q and ns > 0:
                    cost, fn = np_q.pop(0)
                    fn()
                    ns -= cost

            def drain_all():
                while filler_q:
                    filler_q.pop(0)[1]()
                while np_q:
                    np_q.pop(0)[1]()

            # ---------------- attention ----------------
            def attn_chunk(q0, qw):
                kt0 = q0 // 128
                nkt = kt0 + qw // 128   # causal k-tiles for this chunk
                tsl = slice(q0, q0 + qw)
                sums = work.tile([128, TC], F32, tag="sums", name="sums")
                nc.vector.memset(sums[:, :qw], 1.0)
                for p in range(2):          # head pairs (2p, 2p+1)
                    o_ps = [
                        ps_o.tile([128, TC], F32, tag=f"o{h01}", name=f"o{h01}")
                        for h01 in range(2)
                    ]

                    def emit_pv(w, pts):
                        # PV for window w (software-pipelined: emitted one
                        # window late so the PE never waits on this window's
                        # exp)
                        for h01 in range(2):
                            h = 2 * p + h01
                            for kt01 in range(2):
                                kt = 2 * w + kt01
                                qlo = max(0, 128 * (kt - kt0))
                                nc.tensor.matmul(
                                    o_ps[h01][: HD + 1, qlo:qw],
                                    lhsT=v_s[:, kt, h * (HD + 1):(h + 1) * (HD + 1)],
                                    rhs=pts[h01][:, kt01 * qw + qlo:(kt01 + 1) * qw],
                                    start=(kt == 0),
                                    stop=(kt == nkt - 1),
                                )

                    pend = None
                    for w in range(nkt // 2):   # windows of 2 k-tiles
                        j0 = 2 * w - kt0
                        pts = []
                        for h01 in range(2):
                            dsl = slice(64 * h01, 64 * h01 + 64)
                            s_ps = ps_s.tile(
                                [128, 2 * TC], F32, tag=f"s{h01}", name=f"s{h01}",
                                bufs=1,
                            )
                            for kt01 in range(2):
                                kt = 2 * w + kt01
                                # on the 5/8-masked last window only cols the
                                # exp reads are streamed (rest is stale PSUM,
                                # never read)
                                slo = 128 * (j0 + kt01) if (qw == TC and j0 == 2) else 0
                                nc.tensor.matmul(
                                    s_ps[:, kt01 * qw + slo:(kt01 + 1) * qw],
                                    lhsT=qkvt_s[dsl, 2 + p, bass.ts(kt, KT)],
                                    rhs=qkvt_s[dsl, p, slice(q0 + slo, q0 + qw)],
                                    start=True, stop=True,
                                )
                            pt = work.tile(
                                [128, 2 * TC], BF16, tag=f"pt{h01}", name=f"pt{h01}",
                                bufs=6,
                            )
                            pts.append(pt)
                            if qw == TC and j0 == 2:
                                # last window is 5/8 masked: exp only live cols
                                for kt01 in range(2):
                                    qlo = 128 * (j0 + kt01)
                                    nc.scalar.activation(
                                        pt[:, kt01 * qw + qlo:(kt01 + 1) * qw],
                                        s_ps[:, kt01 * qw + qlo:(kt01 + 1) * qw],
                                        mybir.ActivationFunctionType.Exp, scale=0.125,
                                    )
                            else:
                                nc.scalar.activation(
                                    pt[:, :2 * qw], s_ps[:, :2 * qw],
                                    mybir.ActivationFunctionType.Exp, scale=0.125,
                                )
                            for kt01 in range(2):
                                kt = 2 * w + kt01
                                j = kt - kt0
                                if j >= 0:  # diagonal tiles: causal masking,
                                    # band-trimmed: cols < 128j are skipped by
                                    # PV's qlo, cols >= 128(j+1) are unmasked
                                    blo = 128 * j
                                    bhi = min(blo + 128, qw)
                                    nc.gpsimd.tensor_mul(
                                        pt[:, kt01 * qw + blo:kt01 * qw + bhi],
                                        pt[:, kt01 * qw + blo:kt01 * qw + bhi],
                                        mask_s[:, j, blo:bhi],
                                    )
                        if pend is not None:
                            emit_pv(*pend)
                        pend = (w, pts)
                        # PE slack per window: exp pace minus window PE work
                        drain(1100 if qw == TC else 800)
                    emit_pv(*pend)
                    # copy O out unnormalized (frees psum); gather denominators
                    for h01 in range(2):
                        h = 2 * p + h01
                        nc.vector.tensor_copy(
                            ot_s[64 * h01:64 * h01 + 64, p, tsl],
                            o_ps[h01][0:HD, :qw],
                        )
                        nc.vector.tensor_copy(
                            sums[32 * h:32 * h + 1, :qw], o_ps[h01][HD:HD + 1, :qw]
                        )
                    drain(700)
                return sums

            # ---------------- main schedule ----------------
            # minimal prefix: exactly what attention chunk 0 needs
            qk_group(0, 0)   # q pair 0
            qk_group(1, 0)   # q pair 1
            qk_group(2, 0)   # k pair 0
            qk_group(3, 0)   # k pair 1
            v_group(0)
            v_group(1)
            # chunk 1 needs v tokens 256:512; queue as highest-priority filler
            filler_q.append((1000, lambda: v_group(2)))
            filler_q.append((1000, lambda: v_group(3)))

            # per-chunk prerequisite fillers (k/v tiles + q columns)
            prereq = {
                2: [(1800, lambda j=j: qk_group(j, 1)) for j in (2, 3, 0, 1)]
                   + [(1000, lambda tt=tt: v_group(tt)) for tt in range(4, 8)],
                3: [(1800, lambda j=j: qk_group(j, 2)) for j in (2, 3, 0, 1)]
                   + [(1000, lambda tt=tt: v_group(tt)) for tt in range(8, 12)],
                4: [(1800, lambda j=j: qk_group(j, 3)) for j in (2, 3, 0, 1)]
                   + [(1000, lambda tt=tt: v_group(tt)) for tt in range(12, 16)],
            }

            states = []
            for cidx, (q0, qw) in enumerate(CHUNKS):
                allow_np[0] = cidx >= 3
                # queue prereqs of the NEXT chunk at the FRONT so they drain
                # during this chunk ahead of np work
                if cidx + 1 in prereq:
                    filler_q[0:0] = prereq[cidx + 1]
                # safety net: force-emit leftovers this chunk still needs
                if cidx in prereq:
                    remaining = [f for f in filler_q if f in prereq[cidx]]
                    for f in remaining:
                        filler_q.remove(f)
                        f[1]()
                sums = attn_chunk(q0, qw)
                states.append({"q0": q0, "qw": qw, "sums": sums})
                push_np(states[-1])
            allow_np[0] = True
            drain_all()

    return nc


# ---------------- host side ----------------

def _bf(a):
    return np.ascontiguousarray(np.asarray(a, dtype=np.float32).astype(NPBF))


def make_in_maps(inputs):
    x = np.asarray(inputs["x"], np.float32)
    W_attn = np.asarray(inputs["W_attn"], np.float32)
    A_attn = np.asarray(inputs["A_attn"], np.float32)
    B_attn = np.asarray(inputs["B_attn"], np.float32)
    W_proj = np.asarray(inputs["W_proj"], np.float32)
    A_proj = np.asarray(inputs["A_proj"], np.float32)
    B_proj = np.asarray(inputs["B_proj"], np.float32)
    # b_attn / b_proj are zeros per the problem spec; not sent to the device.

    # LoRA folded: x@(W + s*B@A)^T  ==  x@W^T + s*(x@A^T)@B^T  exactly.
    W_attn_eff = W_attn + LORA_SCALE * (B_attn @ A_attn)
    W_proj_eff = W_proj + LORA_SCALE * (B_proj @ A_proj)

    kk = np.arange(KT)[:, None]
    qq = np.arange(TC)[None, :]
    masks = np.stack(
        [(qq >= kk + KT * j).astype(np.float32) for j in range(4)]
    )

    in_maps = []
    for core in range(8):
        b, m = divmod(core, TP)
        rs = slice(OQ * m, OQ * (m + 1))
        w_shard = np.concatenate(
            [W_attn_eff[rs], W_attn_eff[C:][rs], W_attn_eff[2 * C:][rs]], axis=0
        )
        cs = slice(CP * m, CP * (m + 1))
        in_maps.append({
            "xt": _bf(x[b].T),
            "wqkvt": _bf(w_shard.T),
            "wpt": _bf(W_proj_eff[:, cs].T),
            "masks": _bf(masks),
        })
    return in_maps


def assemble(outs):
    y = np.zeros((B, T, C), np.float32)
    for g in range(B):
        yt = np.zeros((C, T), np.float32)
        for r in range(TP):
            yt += np.asarray(outs[TP * g + r], np.float32)
        y[g] = yt.T
    return y


_CACHE = {}


def run(inputs, trace=False):
    from concourse.bass_utils import run_bass_kernel_spmd

    if "nc" not in _CACHE:
        nc = build_nc()
        nc.compile()
        _CACHE["nc"] = nc
    res = run_bass_kernel_spmd(
        _CACHE["nc"], make_in_maps(inputs), core_ids=list(range(8)), trace=trace,
    )
    outs = [r["out"] for r in res.results]
    return assemble(outs), res


def kernel(**inputs):
    y, _ = run(inputs)
    return y


# revision 19
# speedup vs baseline: 1.1470x; 1.1470x over previous
"""Distributed Trainium2 kernel for causal multi-head attention with LoRA
(c_attn + c_proj both LoRA'd), B=2 T=2048 C=1024 H=16 hd=64 r=8.

Sharding: data-parallel over batch (2 groups of 4 cores) x tensor-parallel
over heads (4 heads / core).  Each core computes qkv for its heads, causal
attention, and a partial c_proj over its 256 input dims; the host sums the
4 partial outputs per batch group.

Host-side simplifications (all exact linear algebra, no approximation):
 - LoRA folds into the base weights: W_eff = W + LORA_SCALE * B @ A.
 - Everything is passed feature-major ("pre-transposed") so no on-device
   transposes are needed; the device output is y^T, transposed back on host.
 - b_attn / b_proj are zeros by the problem spec and are not applied.

Device compute is bf16 (fp32 PSUM accumulation; rel-err budget 2e-2).

Schedule: attention chunks ascend (smallest first) so the ScalarE-heavy
softmax work of early chunks overlaps the PE-heavy qkv matmuls of later
k-tiles.  A queue of PE "filler" closures (qkv groups, v groups, c_proj
m-tiles of finished chunks) is drained between attention windows so the PE
never idles long enough for the HAM clock gate to re-throttle.

Engine placement: ScalarE does ONLY exp.  All PSUM->SBUF drains (qkv/v/o/
sums/y copies) are on VectorE.  Causal mask multiplies (SBUF-only, band-
trimmed to the 128-col partial-diagonal band) are on GpSimd.  Softmax
denominators use the augmented-V ones-column trick (no extra PE streams);
their reciprocal is DVE reciprocal_approx_fast.
"""

import numpy as np
import ml_dtypes

import concourse.bass as bass
import concourse.mybir as mybir
import concourse.tile as tile
from concourse import bacc

BF16 = mybir.dt.bfloat16
F32 = mybir.dt.float32
NPBF = ml_dtypes.bfloat16

B, T, C = 2, 2048, 1024
H, HD, R = 16, 64, 8
LORA_SCALE = 2.0

TP = 4                 # tensor-parallel ranks per batch group
HL = H // TP           # heads per core = 4
OQ = HL * HD           # local q rows = 256
OL = 3 * OQ            # local qkv rows = 768
CP = C // TP           # local c_proj contraction dims = 256
TC = 512               # t-chunk (matmul free dim)
NTC = T // TC          # 4 chunks
KT = 128               # k tile (partition dim of S^T)
NCT = C // 128         # 8 contraction tiles for c_attn

# attention chunks, ascending so softmax overlaps remaining qkv work
CHUNKS = [(0, 256), (256, 256), (512, 512), (1024, 512), (1536, 512)]


def build_nc():
    nc = bacc.Bacc(None, target_bir_lowering=False)

    drain_ns = [1800]  # per-window filler budget; set per chunk

    xt_d = nc.declare_dram_parameter("xt", [C, T], BF16, isOutput=False)
    wqkvt_d = nc.declare_dram_parameter("wqkvt", [C, OL], BF16, isOutput=False)
    wpt_d = nc.declare_dram_parameter("wpt", [CP, C], BF16, isOutput=False)
    masks_d = nc.declare_dram_parameter("masks", [4, KT, TC], BF16, isOutput=False)
    out_d = nc.declare_dram_parameter("out", [C, T], BF16, isOutput=True)

    with tile.TileContext(nc) as tc:
        with (
            tc.tile_pool(name="const", bufs=1) as const,
            tc.tile_pool(name="work", bufs=3) as work,
            tc.tile_pool(name="ps_lin", bufs=2, space="PSUM") as ps_lin,
            tc.tile_pool(name="ps_s", bufs=1, space="PSUM") as ps_s,
            tc.tile_pool(name="ps_o", bufs=1, space="PSUM") as ps_o,
        ):
            # ---------------- persistent SBUF tensors ----------------
            # wq[n] and xt[n, ci0] interleaved so the first qk group's n-loop
            # can start as soon as the first pairs land
            wq_s = const.tile([128, NCT, OL], BF16, tag="wq")
            wq_r = wqkvt_d.rearrange("(n p) o -> p n o", p=128)
            xt_s = const.tile([128, NCT, T], BF16, tag="xt")
            xt_r = xt_d.rearrange("(n p) t -> p n t", p=128)
            for n in range(NCT):
                nc.sync.dma_start(out=wq_s[:, n, :], in_=wq_r[:, n, :])
                nc.sync.dma_start(
                    out=xt_s[:, n, bass.ts(0, TC)],
                    in_=xt_r[:, n, bass.ts(0, TC)],
                )

            mask_s = const.tile([128, 4, TC], BF16, tag="mask")
            nc.sync.dma_start(out=mask_s, in_=masks_d.rearrange("j p q -> p j q"))

            for ci in range(1, NTC):
                for n in range(NCT):
                    nc.sync.dma_start(
                        out=xt_s[:, n, bass.ts(ci, TC)],
                        in_=xt_r[:, n, bass.ts(ci, TC)],
                    )

            wpt_s = const.tile([128, CP // 128, C], BF16, tag="wpt")
            nc.sync.dma_start(out=wpt_s, in_=wpt_d.rearrange("(n p) o -> p n o", p=128))

            # q,k feature-major: tiles 0,1 = q (256 rows), 2,3 = k
            qkvt_s = const.tile([128, 4, T], BF16, tag="qkvt")
            # v token-major, augmented: per t-tile, 4 heads x (64 dims + ones)
            v_s = const.tile([128, T // 128, HL * (HD + 1)], BF16, tag="v")
            nc.vector.memset(v_s, 1.0)  # ones columns survive the V copies
            ot_s = const.tile([128, CP // 128, T], BF16, tag="ot")
            ones_s = const.tile([128, 64], BF16, tag="ones")
            nc.vector.memset(ones_s, 1.0)

            # PE warmup: dummy matmuls during the input-DMA window so the
            # HAM clock gate reaches 8/8 before real work starts.  The memset
            # runs on GpSimd (whose queue is otherwise empty at t0) so the
            # warmup isn't blocked behind the DVE memset/TENSOR_LOAD chain.
            warm_s = const.tile([128, TC], BF16, tag="warm")
            nc.gpsimd.memset(warm_s, 0.0)
            warm_ps = ps_lin.tile([128, TC], F32, tag="lin", name="warm_ps")
            for _ in range(26):
                nc.tensor.matmul(
                    warm_ps, lhsT=warm_s[:, :128], rhs=warm_s,
                    start=True, stop=True,
                )

            # ---------------- PE filler emitters ----------------
            def qk_group(j, ci):
                # feature-major q/k: o-tile j (0,1=q pairs; 2,3=k pairs)
                osl = bass.ts(j, 128)
                tsl = bass.ts(ci, TC)
                qk_ps = ps_lin.tile([128, TC], F32, tag="lin", name="qk_ps")
                for n in range(NCT):
                    nc.tensor.matmul(
                        qk_ps, lhsT=wq_s[:, n, osl], rhs=xt_s[:, n, tsl],
                        start=(n == 0), stop=(n == NCT - 1),
                    )
                nc.vector.tensor_copy(qkvt_s[:, j, tsl], qk_ps)

            def v_group(tt):
                # v token-major (+ ones column per head)
                v_ps = ps_lin.tile([128, TC], F32, tag="lin", name="v_ps")
                ttsl = bass.ts(tt, 128)
                for n in range(NCT):
                    nc.tensor.matmul(
                        v_ps[:, :OQ], lhsT=xt_s[:, n, ttsl], rhs=wq_s[:, n, 2 * OQ:OL],
                        start=(n == 0), stop=(n == NCT - 1),
                    )
                dst = v_s[:, tt, :].rearrange("p (h e) -> p h e", e=HD + 1)[:, :, 0:HD]
                nc.vector.tensor_copy(dst, v_ps[:, :OQ].rearrange("p (h e) -> p h e", e=HD))

            # np (normalize + c_proj) sub-steps for a finished chunk
            def np_head(st):
                q0, qw, sums = st["q0"], st["qw"], st["sums"]
                recf = work.tile([128, TC], F32, tag="recf", name="recf")
                recip = work.tile([128, TC], BF16, tag="recip", name="recip")
                nc.vector.reciprocal_approx_fast(recf[:, :qw], sums[:, :qw])
                with nc.allow_low_precision(reason="softmax denom, 2e-2 budget"):
                    nc.vector.tensor_copy(recip[:, :qw], recf[:, :qw])
                tsl = slice(q0, q0 + qw)
                for p in range(2):
                    rb_ps = ps_lin.tile([128, TC], F32, tag="lin", name="rb_ps")
                    for h01 in range(2):
                        h = 2 * p + h01
                        nc.tensor.matmul(
                            rb_ps[64 * h01:64 * h01 + 64, :qw],
                            lhsT=ones_s[32 * h:32 * h + 1, :],
                            rhs=recip[32 * h:32 * h + 1, :qw],
                            start=True, stop=True,
                            tile_position=(32 * h, 64 * h01),
                        )
                    dst = ot_s[:, p, tsl]
                    nc.vector.tensor_mul(dst, dst, rb_ps[:, :qw])
                st["yt"] = work.tile(
                    [128, C // 128, TC], BF16, tag="yt", bufs=2, name="yt"
                )

            def np_y(st, m):
                q0, qw = st["q0"], st["qw"]
                tsl = slice(q0, q0 + qw)
                msl = bass.ts(m, 128)
                y_ps = ps_lin.tile([128, TC], F32, tag="lin", name="y_ps")
                for n in range(CP // 128):
                    nc.tensor.matmul(
                        y_ps[:, :qw], lhsT=wpt_s[:, n, msl], rhs=ot_s[:, n, tsl],
                        start=(n == 0), stop=(n == CP // 128 - 1),
                    )
                nc.vector.tensor_copy(st["yt"][:, m, :qw], y_ps[:, :qw])

            def np_dma(st, half):
                q0, qw = st["q0"], st["qw"]
                tsl = slice(q0, q0 + qw)
                out_r = out_d.rearrange("(m p) t -> p m t", p=128)
                nc.sync.dma_start(
                    out=out_r[:, 4 * half:4 * half + 4, tsl],
                    in_=st["yt"][:, 4 * half:4 * half + 4, :qw],
                )

            # Two filler queues: prereqs of upcoming attention chunks (gate
            # progress, drain first) and np work of finished chunks (no
            # downstream consumer; held until the late chunks where the PE
            # has slack inside exp-paced windows).
            filler_q = []
            np_q = []
            allow_np = [False]

            def push_np(st):
                np_q.append((900, lambda: np_head(st)))
                for m in range(C // 128):
                    np_q.append((500, lambda m=m: np_y(st, m)))
                np_q.append((200, lambda: np_dma(st, 0)))
                np_q.append((200, lambda: np_dma(st, 1)))

            def drain(ns):
                # emit fillers worth ~ns of PE time
                while filler_q and ns > 0:
                    cost, fn = filler_q.pop(0)
                    fn()
                    ns -= cost
                while allow_np[0] and np_q and ns > 0:
                    cost, fn = np_q.pop(0)
                    fn()
                    ns -= cost

            def drain_all():
                while filler_q:
                    filler_q.pop(0)[1]()
                while np_q:
                    np_q.pop(0)[1]()

            # ---------------- attention ----------------
            def attn_chunk(q0, qw):
                kt0 = q0 // 128
                nkt = kt0 + qw // 128   # causal k-tiles for this chunk
                tsl = slice(q0, q0 + qw)
                sums = work.tile([128, TC], F32, tag="sums", name="sums")
                nc.vector.memset(sums[:, :qw], 1.0)
                for p in range(2):          # head pairs (2p, 2p+1)
                    o_ps = [
                        ps_o.tile([128, TC], F32, tag=f"o{h01}", name=f"o{h01}")
                        for h01 in range(2)
                    ]

                    def emit_pv(w, h01, pt):
                        h = 2 * p + h01
                        for kt01 in range(2):
                            kt = 2 * w + kt01
                            qlo = max(0, 128 * (kt - kt0))
                            nc.tensor.matmul(
                                o_ps[h01][: HD + 1, qlo:qw],
                                lhsT=v_s[:, kt, h * (HD + 1):(h + 1) * (HD + 1)],
                                rhs=pt[:, kt01 * qw + qlo:(kt01 + 1) * qw],
                                start=(kt == 0),
                                stop=(kt == nkt - 1),
                            )

                    # Cyclic steady-state order keeping ScalarE saturated:
                    #   PV(h0,w-1), S(h0,w), exp(h0,w), PV(h1,w-1), S(h1,w),
                    #   exp(h1,w), fillers
                    # Each PE op's dependency is satisfied exactly when the
                    # in-order PE stream reaches it; exp(h0,w) finishes S-wise
                    # prerequisites ~950ns into exp(h1,w-1)'s 1147ns.
                    pend = [None, None]     # pt of window w-1 per head
                    for w in range(nkt // 2):   # windows of 2 k-tiles
                        j0 = 2 * w - kt0
                        for h01 in range(2):
                            if pend[h01] is not None:
                                emit_pv(w - 1, h01, pend[h01])
                            dsl = slice(64 * h01, 64 * h01 + 64)
                            s_ps = ps_s.tile(
                                [128, 2 * TC], F32, tag=f"s{h01}", name=f"s{h01}",
                                bufs=1,
                            )
                            for kt01 in range(2):
                                kt = 2 * w + kt01
                                # on the 5/8-masked last window only cols the
                                # exp reads are streamed (rest is stale PSUM,
                                # never read)
                                slo = 128 * (j0 + kt01) if (qw == TC and j0 == 2) else 0
                                nc.tensor.matmul(
                                    s_ps[:, kt01 * qw + slo:(kt01 + 1) * qw],
                                    lhsT=qkvt_s[dsl, 2 + p, bass.ts(kt, KT)],
                                    rhs=qkvt_s[dsl, p, slice(q0 + slo, q0 + qw)],
                                    start=True, stop=True,
                                )
                            pt = work.tile(
                                [128, 2 * TC], BF16, tag=f"pt{h01}", name=f"pt{h01}",
                                bufs=6,
                            )
                            pend[h01] = pt
                            if qw == TC and j0 == 2:
                                # last window is 5/8 masked: exp only live cols
                                for kt01 in range(2):
                                    qlo = 128 * (j0 + kt01)
                                    nc.scalar.activation(
                                        pt[:, kt01 * qw + qlo:(kt01 + 1) * qw],
                                        s_ps[:, kt01 * qw + qlo:(kt01 + 1) * qw],
                                        mybir.ActivationFunctionType.Exp, scale=0.125,
                                    )
                            else:
                                nc.scalar.activation(
                                    pt[:, :2 * qw], s_ps[:, :2 * qw],
                                    mybir.ActivationFunctionType.Exp, scale=0.125,
                                )
                            for kt01 in range(2):
                                kt = 2 * w + kt01
                                j = kt - kt0
                                if j >= 0:  # diagonal tiles: causal masking,
                                    # band-trimmed: cols < 128j are skipped by
                                    # PV's qlo, cols >= 128(j+1) are unmasked
                                    blo = 128 * j
                                    bhi = min(blo + 128, qw)
                                    nc.gpsimd.tensor_mul(
                                        pt[:, kt01 * qw + blo:kt01 * qw + bhi],
                                        pt[:, kt01 * qw + blo:kt01 * qw + bhi],
                                        mask_s[:, j, blo:bhi],
                                    )
                        drain(drain_ns[0])
                    for h01 in range(2):
                        emit_pv(nkt // 2 - 1, h01, pend[h01])
                    # copy O out unnormalized (frees psum); gather denominators
                    for h01 in range(2):
                        h = 2 * p + h01
                        nc.vector.tensor_copy(
                            ot_s[64 * h01:64 * h01 + 64, p, tsl],
                            o_ps[h01][0:HD, :qw],
                        )
                        nc.vector.tensor_copy(
                            sums[32 * h:32 * h + 1, :qw], o_ps[h01][HD:HD + 1, :qw]
                        )
                    drain(1400)
                return sums

            # ---------------- main schedule ----------------
            # minimal prefix: exactly what attention chunk 0 needs
            qk_group(0, 0)   # q pair 0
            qk_group(1, 0)   # q pair 1
            qk_group(2, 0)   # k pair 0
            qk_group(3, 0)   # k pair 1
            v_group(0)
            v_group(1)
            # chunk 1 needs v tokens 256:512; queue as highest-priority filler
            filler_q.append((1000, lambda: v_group(2)))
            filler_q.append((1000, lambda: v_group(3)))

            # per-chunk prerequisite fillers (k/v tiles + q columns)
            prereq = {
                2: [(1800, lambda j=j: qk_group(j, 1)) for j in (2, 3, 0, 1)]
                   + [(1000, lambda tt=tt: v_group(tt)) for tt in range(4, 8)],
                3: [(1800, lambda j=j: qk_group(j, 2)) for j in (2, 3, 0, 1)]
                   + [(1000, lambda tt=tt: v_group(tt)) for tt in range(8, 12)],
                4: [(1800, lambda j=j: qk_group(j, 3)) for j in (2, 3, 0, 1)]
                   + [(1000, lambda tt=tt: v_group(tt)) for tt in range(12, 16)],
            }

            states = []
            for cidx, (q0, qw) in enumerate(CHUNKS):
                allow_np[0] = cidx >= 2
                # early chunks are PE-bound (ScalarE can't saturate yet):
                # drain aggressively; late chunks are exp-paced: ~700ns of
                # PE slack per window
                drain_ns[0] = 1800 if cidx < 3 else 700
                # queue prereqs of the NEXT chunk at the FRONT so they drain
                # during this chunk ahead of np work
                if cidx + 1 in prereq:
                    filler_q[0:0] = prereq[cidx + 1]
                # safety net: force-emit leftovers this chunk still needs
                if cidx in prereq:
                    remaining = [f for f in filler_q if f in prereq[cidx]]
                    for f in remaining:
                        filler_q.remove(f)
                        f[1]()
                sums = attn_chunk(q0, qw)
                states.append({"q0": q0, "qw": qw, "sums": sums})
                push_np(states[-1])
            allow_np[0] = True
            drain_all()

    return nc


# ---------------- host side ----------------

def _bf(a):
    return np.ascontiguousarray(np.asarray(a, dtype=np.float32).astype(NPBF))


def make_in_maps(inputs):
    x = np.asarray(inputs["x"], np.float32)
    W_attn = np.asarray(inputs["W_attn"], np.float32)
    A_attn = np.asarray(inputs["A_attn"], np.float32)
    B_attn = np.asarray(inputs["B_attn"], np.float32)
    W_proj = np.asarray(inputs["W_proj"], np.float32)
    A_proj = np.asarray(inputs["A_proj"], np.float32)
    B_proj = np.asarray(inputs["B_proj"], np.float32)
    # b_attn / b_proj are zeros per the problem spec; not sent to the device.

    # LoRA folded: x@(W + s*B@A)^T  ==  x@W^T + s*(x@A^T)@B^T  exactly.
    W_attn_eff = W_attn + LORA_SCALE * (B_attn @ A_attn)
    W_proj_eff = W_proj + LORA_SCALE * (B_proj @ A_proj)

    kk = np.arange(KT)[:, None]
    qq = np.arange(TC)[None, :]
    masks = np.stack(
        [(qq >= kk + KT * j).astype(np.float32) for j in range(4)]
    )

    in_maps = []
    for core in range(8):
        b, m = divmod(core, TP)
        rs = slice(OQ * m, OQ * (m + 1))
        w_shard = np.concatenate(
            [W_attn_eff[rs], W_attn_eff[C:][rs], W_attn_eff[2 * C:][rs]], axis=0
        )
        cs = slice(CP * m, CP * (m + 1))
        in_maps.append({
            "xt": _bf(x[b].T),
            "wqkvt": _bf(w_shard.T),
            "wpt": _bf(W_proj_eff[:, cs].T),
            "masks": _bf(masks),
        })
    return in_maps


def assemble(outs):
    y = np.zeros((B, T, C), np.float32)
    for g in range(B):
        yt = np.zeros((C, T), np.float32)
        for r in range(TP):
            yt += np.asarray(outs[TP * g + r], np.float32)
        y[g] = yt.T
    return y


_CACHE = {}


def run(inputs, trace=False):
    from concourse.bass_utils import run_bass_kernel_spmd

    if "nc" not in _CACHE:
        nc = build_nc()
        nc.compile()
        _CACHE["nc"] = nc
    res = run_bass_kernel_spmd(
        _CACHE["nc"], make_in_maps(inputs), core_ids=list(range(8)), trace=trace,
    )
    outs = [r["out"] for r in res.results]
    return assemble(outs), res


def kernel(**inputs):
    y, _ = run(inputs)
    return y
